# revision 7
# baseline (speedup 1.0000x reference)
"""Trainium2 Bass kernel for nn_AttnBFAN (batched attention w/ focal re-norm).

Data-parallel over the batch dim: 128 batches sharded 16-per-core across 8
NeuronCores. Per batch (Q=128, C=1024, D=1024):
    attn = leaky_relu(context @ query^T, 0.1)          (C, Q)
    attn = attn / (||attn||_2 over q)                  l2norm per (b, c)
    p    = softmax(20 * attn^T, axis=c)                (Q, C)
    t    = (p > mean_c p) * p ; re_attn = t / sum_c t
    wcontext = re_attn @ context                       (Q, D)
returns (query, wcontext, re_attn).

On-chip layout: the softmax/focal chain lives in (q=partitions, c=free) so all
row reductions are free-axis; the l2 norm (over q) uses a ones-matmul on the PE
which also broadcasts the result to all 128 partitions. Matmuls run in f32r
(full-rate fp32 streaming, ~2^-13 rounding); context is loaded once per batch
and transposed on the PE (f32r transpose mode) for the first bmm.
"""

import os
import numpy as np

import concourse.bacc as bacc
import concourse.mybir as mybir
import concourse.tile as tile
from concourse.bass_utils import run_bass_kernel_spmd
from concourse.masks import make_identity
from concourse.hw_specs import get_activation_tables

F32 = mybir.dt.float32
F32R = mybir.dt.float32r
AX = mybir.AxisListType
ALU = mybir.AluOpType
ACTF = mybir.ActivationFunctionType

NCORES = 8
NB = 128          # total batches
BPC = NB // NCORES  # batches per core
Q = 128
C = 1024
D = 1024
SMOOTH = 20.0

_CACHE = {}


def _build():
    nc = bacc.Bacc("TRN2", target_bir_lowering=False, debug=False,
                   num_devices=NCORES, name="attn_bfan")
    q_in = nc.dram_tensor("query", [BPC, Q, D], F32, kind="ExternalInput")
    c_in = nc.dram_tensor("context", [BPC, C, D], F32, kind="ExternalInput")
    re_out = nc.dram_tensor("re_attn", [BPC, Q, C], F32, kind="ExternalOutput")
    wc_out = nc.dram_tensor("wcontext", [BPC, Q, D], F32, kind="ExternalOutput")

    with tile.TileContext(nc) as tc:
        with (
            tc.tile_pool(name="singles", bufs=1) as singles,
            tc.tile_pool(name="ctxp", bufs=3) as ctxp,
            tc.tile_pool(name="ctxtp", bufs=1) as ctxtp,
            tc.tile_pool(name="qp", bufs=2) as qp,
            tc.tile_pool(name="work", bufs=2) as work,
            tc.tile_pool(name="w1", bufs=1) as w1,
            tc.tile_pool(name="stat", bufs=2) as stat,
            tc.tile_pool(name="ps_w", bufs=1, space="PSUM") as ps_w,
            tc.tile_pool(name="ps_f", bufs=2, space="PSUM") as ps_f,
            tc.tile_pool(name="ps_tp", bufs=2, space="PSUM") as ps_tp,
        ):
            tab_names = list(get_activation_tables("gen3").keys())
            nc.scalar.add_instruction(mybir.InstLoadActFuncSet(
                name=nc.get_next_instruction_name(),
                act_func_set_id=tab_names.index("natural_log_exp_and_others"),
                ins=[], outs=[]))
            ident = singles.tile([128, 128], F32, tag="ident")
            make_identity(nc, ident[:])
            identr = singles.tile([128, 128], F32R, tag="identr")
            nc.vector.tensor_copy(identr[:], ident[:])
            ones_f = singles.tile([128, 128], F32, tag="ones_f")
            nc.vector.memset(ones_f[:], 1.0)
            ones_r = singles.tile([128, 128], F32R, tag="ones_r")
            nc.vector.tensor_copy(ones_r[:], ones_f[:])
            ln20 = singles.tile([128, 1], F32, tag="ln20")
            nc.vector.memset(ln20[:], float(np.log(SMOOTH)))

            for b in range(BPC):
                # ---- loads (SWDGE cast f32 -> f32r) ----
                ctx = ctxp.tile([128, 8, D], F32R, tag="ctx")
                ctx_src = c_in[b].rearrange("(jc p) d -> p jc d", p=128)
                nc.gpsimd.dma_start(out=ctx[:, 0:4, :], in_=ctx_src[:, 0:4, :])
                nc.gpsimd.dma_start(out=ctx[:, 4:8, :], in_=ctx_src[:, 4:8, :])
                qr = qp.tile([128, D], F32R, tag="qr")
                nc.gpsimd.dma_start(out=qr[:], in_=q_in[b])

                # ---- query^T: 8 PE transposes, 4 per psum bank ----
                qT = qp.tile([128, 8, 128], F32R, tag="qT")
                tp = ps_tp.tile([128, 1024], F32R, tag="tp")
                for jd in range(8):
                    nc.tensor.transpose(
                        tp[:, jd * 128:(jd + 1) * 128],
                        qr[:, jd * 128:(jd + 1) * 128], identr[:])
                nc.vector.tensor_copy(qT[:].rearrange("p a b -> p (a b)"), tp[:])

                # ---- context^T: 64 PE transposes, 4 per bank, 16 copies ----
                ctxT = ctxtp.tile([128, 8, C], F32R, tag="ctxT")
                for jd in range(8):
                    tp = ps_tp.tile([128, 1024], F32R, tag="tp")
                    for jc in range(8):
                        nc.tensor.transpose(
                            tp[:, jc * 128:(jc + 1) * 128],
                            ctx[:, jc, jd * 128:(jd + 1) * 128], identr[:])
                    if jd % 8 < 5:
                        nc.vector.tensor_copy(ctxT[:, jd, :], tp[:])
                    else:
                        nc.scalar.copy(ctxT[:, jd, :], tp[:])

                # ---- bmm1: attn^T (q, c) accumulated over 8 d-chunks ----
                a0 = ps_f.tile([128, 512], F32, tag="tpf")
                a1 = ps_f.tile([128, 512], F32, tag="tpf")
                for jd in range(8):
                    st, sp = jd == 0, jd == 7
                    nc.tensor.matmul(a0[:], qT[:, jd, :], ctxT[:, jd, 0:512],
                                     start=st, stop=sp)
                    nc.tensor.matmul(a1[:], qT[:, jd, :], ctxT[:, jd, 512:1024],
                                     start=st, stop=sp)

                # ---- leaky relu via Prelu(alpha=0.1), PSUM -> SBUF ----
                attn = work.tile([128, C], F32, tag="attn")
                nc.scalar.activation(attn[:, 0:512], a0[:], ACTF.Prelu,
                                     bias=0.0, scale=1.0, alpha=0.1)
                nc.scalar.activation(attn[:, 512:1024], a1[:], ACTF.Prelu,
                                     bias=0.0, scale=1.0, alpha=0.1)

                # ---- l2 norm over q: square, ones-matmul (sums over q AND
                #      broadcasts to 128 partitions), sqrt, reciprocal ----
                sq = w1.tile([128, C], F32R, tag="w1a")
                nc.scalar.square(sq[:], attn[:])
                s0 = ps_f.tile([128, 512], F32, tag="tpf")
                s1 = ps_f.tile([128, 512], F32, tag="tpf")
                nc.tensor.matmul(s0[:], ones_r[:], sq[:, 0:512], start=True, stop=True)
                nc.tensor.matmul(s1[:], ones_r[:], sq[:, 512:1024], start=True, stop=True)
                # 20/sqrt(S) = exp(-0.5*ln(S) + ln 20): keeps ACT on the
                # natural_log_exp table (Exp/Ln/Prelu/Square/Copy), no reloads
                lnS = w1.tile([128, C], F32, tag="w1b")
                nc.scalar.activation(lnS[:, 0:512], s0[:], ACTF.Ln)
                nc.scalar.activation(lnS[:, 512:1024], s1[:], ACTF.Ln)
                rn20 = w1.tile([128, C], F32, tag="w1c")
                nc.scalar.activation(rn20[:], lnS[:], ACTF.Exp,
                                     bias=ln20[:], scale=-0.5)
                u = w1.tile([128, C], F32, tag="w1a")
                nc.vector.tensor_mul(u[:], attn[:], rn20[:])

                # ---- softmax (no max-sub; |u| <= 20) with fused row-sum ----
                pu = work.tile([128, C], F32, tag="pu")
                rs = stat.tile([128, 1], F32, tag="rs")
                nc.scalar.activation(pu[:], u[:], ACTF.Exp,
                                     bias=0.0, scale=1.0, accum_out=rs[:])

                # ---- focal threshold: t = (pu > rs/C) * pu, ts = sum_c t ----
                thr = stat.tile([128, 1], F32, tag="thr")
                nc.scalar.mul(thr[:], rs[:], 1.0 / C)
                t = w1.tile([128, C], F32, tag="w1b")
                ts = stat.tile([128, 1], F32, tag="ts")
                nc.vector.scalar_tensor_tensor(
                    out=t[:], in0=pu[:], scalar=thr[:], in1=pu[:],
                    op0=ALU.is_gt, op1=ALU.mult, accum_out=ts[:])
                rinv = stat.tile([128, 1], F32, tag="rinv")
                nc.vector.reciprocal(rinv[:], ts[:])
                re = work.tile([128, C], F32, tag="re")
                nc.scalar.activation(re[:], t[:], ACTF.Copy, bias=0.0, scale=rinv[:])
                nc.sync.dma_start(out=re_out[b], in_=re[:])

                # ---- re_attn^T for bmm2 (f32 transpose, rounded on eviction) ----
                raT = w1.tile([128, 8, 128], F32R, tag="raT")
                for g in range(2):
                    tpf = ps_f.tile([128, 512], F32, tag="tpf")
                    for j in range(4):
                        jc = g * 4 + j
                        nc.tensor.transpose(
                            tpf[:, j * 128:(j + 1) * 128],
                            re[:, jc * 128:(jc + 1) * 128], ident[:])
                    if g == 0:
                        nc.vector.tensor_copy(
                            raT[:, 0:4, :].rearrange("p a b -> p (a b)"), tpf[:])
                    else:
                        nc.scalar.copy(
                            raT[:, 4:8, :].rearrange("p a b -> p (a b)"), tpf[:])

                # ---- bmm2: wcontext (q, d) accumulated over 8 c-chunks ----
                w0 = ps_w.tile([128, 512], F32, tag="w0")
                w2 = ps_w.tile([128, 512], F32, tag="w1")
                for jc in range(8):
                    st, sp = jc == 0, jc == 7
                    nc.tensor.matmul(w0[:], raT[:, jc, :], ctx[:, jc, 0:512],
                                     start=st, stop=sp)
                    nc.tensor.matmul(w2[:], raT[:, jc, :], ctx[:, jc, 512:1024],
                                     start=st, stop=sp)
                wc = work.tile([128, D], F32, tag="wc")
                nc.scalar.copy(wc[:, 0:512], w0[:])
                nc.vector.tensor_copy(wc[:, 512:1024], w2[:])
                nc.sync.dma_start(out=wc_out[b], in_=wc[:])

    nc.compile()
    return nc


def kernel(query: np.ndarray, context: np.ndarray):
    query = np.ascontiguousarray(query, dtype=np.float32)
    context = np.ascontiguousarray(context, dtype=np.float32)
    assert query.shape == (NB, Q, D) and context.shape == (NB, C, D)

    if "nc" not in _CACHE:
        _CACHE["nc"] = _build()
    nc = _CACHE["nc"]

    in_maps = []
    for k in range(NCORES):
        sl = slice(k * BPC, (k + 1) * BPC)
        in_maps.append({"query": query[sl], "context": context[sl]})

    trace = os.environ.get("KERNEL_TRACE", "0") == "1"
    res = run_bass_kernel_spmd(nc, in_maps, core_ids=list(range(NCORES)),
                               trace=trace)
    _CACHE["last_res"] = res

    re_attn = np.concatenate([r["re_attn"] for r in res.results], axis=0)
    wcontext = np.concatenate([r["wcontext"] for r in res.results], axis=0)
    return query, wcontext, re_attn


# revision 8
# speedup vs baseline: 1.2195x; 1.2195x over previous
"""Trainium2 Bass kernel for nn_AttnBFAN (batched attention w/ focal re-norm).

Data-parallel over the batch dim: 128 batches sharded 16-per-core across 8
NeuronCores. Per batch (Q=128, C=1024, D=1024):
    attn = leaky_relu(context @ query^T, 0.1)          (C, Q)
    attn = attn / (||attn||_2 over q)                  l2norm per (b, c)
    p    = softmax(20 * attn^T, axis=c)                (Q, C)
    t    = (p > mean_c p) * p ; re_attn = t / sum_c t
    wcontext = re_attn @ context                       (Q, D)
returns (query, wcontext, re_attn).

On-chip layout: the softmax/focal chain lives in (q=partitions, c=free) so all
row reductions are free-axis; the l2 norm (over q) uses a ones-matmul on the PE
which also broadcasts the result to all 128 partitions. Matmuls run in f32r
(full-rate fp32 streaming, ~2^-13 rounding); context is loaded once per batch
and transposed on the PE (f32r transpose mode) for the first bmm.
"""

import os
import numpy as np

import concourse.bacc as bacc
import concourse.mybir as mybir
import concourse.tile as tile
from concourse.bass_utils import run_bass_kernel_spmd
from concourse.masks import make_identity
from concourse.hw_specs import get_activation_tables

F32 = mybir.dt.float32
F32R = mybir.dt.float32r
AX = mybir.AxisListType
ALU = mybir.AluOpType
ACTF = mybir.ActivationFunctionType

NCORES = 8
NB = 128          # total batches
BPC = NB // NCORES  # batches per core
Q = 128
C = 1024
D = 1024
SMOOTH = 20.0

_CACHE = {}


def _build():
    nc = bacc.Bacc("TRN2", target_bir_lowering=False, debug=False,
                   num_devices=NCORES, name="attn_bfan")
    q_in = nc.dram_tensor("query", [BPC, Q, D], F32, kind="ExternalInput")
    c_in = nc.dram_tensor("context", [BPC, C, D], F32, kind="ExternalInput")
    re_out = nc.dram_tensor("re_attn", [BPC, Q, C], F32, kind="ExternalOutput")
    wc_out = nc.dram_tensor("wcontext", [BPC, Q, D], F32, kind="ExternalOutput")

    with tile.TileContext(nc) as tc:
        with (
            tc.tile_pool(name="singles", bufs=1) as singles,
            tc.tile_pool(name="ctxp", bufs=3) as ctxp,
            tc.tile_pool(name="ctxtp", bufs=1) as ctxtp,
            tc.tile_pool(name="qp", bufs=2) as qp,
            tc.tile_pool(name="work", bufs=2) as work,
            tc.tile_pool(name="w1", bufs=1) as w1,
            tc.tile_pool(name="stat", bufs=2) as stat,
            tc.tile_pool(name="ps_a", bufs=1, space="PSUM") as ps_a,
            tc.tile_pool(name="ps_w", bufs=1, space="PSUM") as ps_w,
            tc.tile_pool(name="ps_f", bufs=2, space="PSUM") as ps_f,
            tc.tile_pool(name="ps_tp", bufs=2, space="PSUM") as ps_tp,
        ):
            tab_names = list(get_activation_tables("gen3").keys())
            nc.scalar.add_instruction(mybir.InstLoadActFuncSet(
                name=nc.get_next_instruction_name(),
                act_func_set_id=tab_names.index("natural_log_exp_and_others"),
                ins=[], outs=[]))
            ident = singles.tile([128, 128], F32, tag="ident")
            make_identity(nc, ident[:])
            identr = singles.tile([128, 128], F32R, tag="identr")
            nc.vector.tensor_copy(identr[:], ident[:])
            ones_f = singles.tile([128, 128], F32, tag="ones_f")
            nc.vector.memset(ones_f[:], 1.0)
            ones_r = singles.tile([128, 128], F32R, tag="ones_r")
            nc.vector.tensor_copy(ones_r[:], ones_f[:])
            ln20 = singles.tile([128, 1], F32, tag="ln20")
            nc.vector.memset(ln20[:], float(np.log(SMOOTH)))

            for b in range(BPC):
                # ---- loads (SWDGE cast f32 -> f32r) ----
                ctx = ctxp.tile([128, 8, D], F32R, tag="ctx")
                ctx_src = c_in[b].rearrange("(jc p) d -> p jc d", p=128)
                nc.gpsimd.dma_start(out=ctx[:, 0:4, :], in_=ctx_src[:, 0:4, :])
                nc.gpsimd.dma_start(out=ctx[:, 4:8, :], in_=ctx_src[:, 4:8, :])
                qr = qp.tile([128, D], F32R, tag="qr")
                nc.gpsimd.dma_start(out=qr[:], in_=q_in[b])

                # ---- query^T: 8 PE transposes, 4 per psum bank ----
                qT = qp.tile([128, 8, 128], F32R, tag="qT")
                for g in range(2):
                    tp = ps_tp.tile([128, 512], F32R, tag="tp")
                    for j in range(4):
                        jd = g * 4 + j
                        nc.tensor.transpose(
                            tp[:, j * 128:(j + 1) * 128],
                            qr[:, jd * 128:(jd + 1) * 128], identr[:])
                    if g == 0:
                        nc.vector.tensor_copy(
                            qT[:, 0:4, :].rearrange("p a b -> p (a b)"), tp[:])
                    else:
                        nc.scalar.copy(
                            qT[:, 4:8, :].rearrange("p a b -> p (a b)"), tp[:])

                # ---- context^T: 64 PE transposes, 4 per bank, 16 copies ----
                ctxT = ctxtp.tile([128, 8, C], F32R, tag="ctxT")
                for jd in range(8):
                    for g in range(2):
                        tp = ps_tp.tile([128, 512], F32R, tag="tp")
                        for j in range(4):
                            jc = g * 4 + j
                            nc.tensor.transpose(
                                tp[:, j * 128:(j + 1) * 128],
                                ctx[:, jc, jd * 128:(jd + 1) * 128], identr[:])
                        if (jd * 2 + g) % 8 < 5:
                            nc.vector.tensor_copy(ctxT[:, jd, g * 512:(g + 1) * 512], tp[:])
                        else:
                            nc.scalar.copy(ctxT[:, jd, g * 512:(g + 1) * 512], tp[:])

                # ---- bmm1: attn^T (q, c) accumulated over 8 d-chunks ----
                a0 = ps_a.tile([128, 512], F32, tag="a0")
                a1 = ps_a.tile([128, 512], F32, tag="a1")
                for jd in range(8):
                    st, sp = jd == 0, jd == 7
                    nc.tensor.matmul(a0[:], qT[:, jd, :], ctxT[:, jd, 0:512],
                                     start=st, stop=sp)
                    nc.tensor.matmul(a1[:], qT[:, jd, :], ctxT[:, jd, 512:1024],
                                     start=st, stop=sp)

                # ---- leaky relu via Prelu(alpha=0.1), PSUM -> SBUF ----
                attn = work.tile([128, C], F32, tag="attn")
                nc.scalar.activation(attn[:, 0:512], a0[:], ACTF.Prelu,
                                     bias=0.0, scale=1.0, alpha=0.1)
                nc.scalar.activation(attn[:, 512:1024], a1[:], ACTF.Prelu,
                                     bias=0.0, scale=1.0, alpha=0.1)

                # ---- l2 norm over q: square, ones-matmul (sums over q AND
                #      broadcasts to 128 partitions), sqrt, reciprocal ----
                sq = w1.tile([128, C], F32R, tag="w1a")
                nc.scalar.square(sq[:], attn[:])
                s0 = ps_f.tile([128, 512], F32, tag="tpf")
                s1 = ps_f.tile([128, 512], F32, tag="tpf")
                nc.tensor.matmul(s0[:], ones_r[:], sq[:, 0:512], start=True, stop=True)
                nc.tensor.matmul(s1[:], ones_r[:], sq[:, 512:1024], start=True, stop=True)
                # 20/sqrt(S) = exp(-0.5*ln(S) + ln 20): keeps ACT on the
                # natural_log_exp table (Exp/Ln/Prelu/Square/Copy), no reloads
                lnS = w1.tile([128, C], F32, tag="w1b")
                nc.scalar.activation(lnS[:, 0:512], s0[:], ACTF.Ln)
                nc.scalar.activation(lnS[:, 512:1024], s1[:], ACTF.Ln)
                rn20 = w1.tile([128, C], F32, tag="w1c")
                nc.scalar.activation(rn20[:], lnS[:], ACTF.Exp,
                                     bias=ln20[:], scale=-0.5)
                u = w1.tile([128, C], F32, tag="w1a")
                nc.vector.tensor_mul(u[:], attn[:], rn20[:])

                # ---- softmax (no max-sub; |u| <= 20) with fused row-sum ----
                pu = work.tile([128, C], F32, tag="pu")
                rs = stat.tile([128, 1], F32, tag="rs")
                nc.scalar.activation(pu[:], u[:], ACTF.Exp,
                                     bias=0.0, scale=1.0, accum_out=rs[:])

                # ---- focal threshold: t = (pu > rs/C) * pu, ts = sum_c t ----
                thr = stat.tile([128, 1], F32, tag="thr")
                nc.scalar.mul(thr[:], rs[:], 1.0 / C)
                t = w1.tile([128, C], F32, tag="w1b")
                ts = stat.tile([128, 1], F32, tag="ts")
                nc.vector.scalar_tensor_tensor(
                    out=t[:], in0=pu[:], scalar=thr[:], in1=pu[:],
                    op0=ALU.is_gt, op1=ALU.mult, accum_out=ts[:])
                rinv = stat.tile([128, 1], F32, tag="rinv")
                nc.vector.reciprocal(rinv[:], ts[:])
                re = work.tile([128, C], F32, tag="re")
                nc.scalar.activation(re[:], t[:], ACTF.Copy, bias=0.0, scale=rinv[:])
                nc.sync.dma_start(out=re_out[b], in_=re[:])

                # ---- re_attn^T for bmm2 (f32 transpose, rounded on eviction) ----
                raT = w1.tile([128, 8, 128], F32R, tag="raT")
                for g in range(2):
                    tpf = ps_f.tile([128, 512], F32, tag="tpf")
                    for j in range(4):
                        jc = g * 4 + j
                        nc.tensor.transpose(
                            tpf[:, j * 128:(j + 1) * 128],
                            re[:, jc * 128:(jc + 1) * 128], ident[:])
                    if g == 0:
                        nc.vector.tensor_copy(
                            raT[:, 0:4, :].rearrange("p a b -> p (a b)"), tpf[:])
                    else:
                        nc.scalar.copy(
                            raT[:, 4:8, :].rearrange("p a b -> p (a b)"), tpf[:])

                # ---- bmm2: wcontext (q, d) accumulated over 8 c-chunks ----
                w0 = ps_w.tile([128, 512], F32, tag="w0")
                w2 = ps_w.tile([128, 512], F32, tag="w1")
                for jc in range(8):
                    st, sp = jc == 0, jc == 7
                    nc.tensor.matmul(w0[:], raT[:, jc, :], ctx[:, jc, 0:512],
                                     start=st, stop=sp)
                    nc.tensor.matmul(w2[:], raT[:, jc, :], ctx[:, jc, 512:1024],
                                     start=st, stop=sp)
                wc = work.tile([128, D], F32, tag="wc")
                nc.scalar.copy(wc[:, 0:512], w0[:])
                nc.vector.tensor_copy(wc[:, 512:1024], w2[:])
                nc.sync.dma_start(out=wc_out[b], in_=wc[:])

    nc.compile()
    return nc


def kernel(query: np.ndarray, context: np.ndarray):
    query = np.ascontiguousarray(query, dtype=np.float32)
    context = np.ascontiguousarray(context, dtype=np.float32)
    assert query.shape == (NB, Q, D) and context.shape == (NB, C, D)

    if "nc" not in _CACHE:
        _CACHE["nc"] = _build()
    nc = _CACHE["nc"]

    in_maps = []
    for k in range(NCORES):
        sl = slice(k * BPC, (k + 1) * BPC)
        in_maps.append({"query": query[sl], "context": context[sl]})

    trace = os.environ.get("KERNEL_TRACE", "0") == "1"
    res = run_bass_kernel_spmd(nc, in_maps, core_ids=list(range(NCORES)),
                               trace=trace)
    _CACHE["last_res"] = res

    re_attn = np.concatenate([r["re_attn"] for r in res.results], axis=0)
    wcontext = np.concatenate([r["wcontext"] for r in res.results], axis=0)
    return query, wcontext, re_attn


# revision 9
# speedup vs baseline: 1.2680x; 1.0397x over previous
"""Trainium2 Bass kernel for nn_AttnBFAN (batched attention w/ focal re-norm).

Data-parallel over the batch dim: 128 batches sharded 16-per-core across 8
NeuronCores. Per batch (Q=128, C=1024, D=1024):
    attn = leaky_relu(context @ query^T, 0.1)          (C, Q)
    attn = attn / (||attn||_2 over q)                  l2norm per (b, c)
    p    = softmax(20 * attn^T, axis=c)                (Q, C)
    t    = (p > mean_c p) * p ; re_attn = t / sum_c t
    wcontext = re_attn @ context                       (Q, D)
returns (query, wcontext, re_attn).

On-chip layout: the softmax/focal chain lives in (q=partitions, c=free) so all
row reductions are free-axis; the l2 norm (over q) uses a ones-matmul on the PE
which also broadcasts the result to all 128 partitions. Matmuls run in f32r
(full-rate fp32 streaming, ~2^-13 rounding); context is loaded once per batch
and transposed on the PE (f32r transpose mode) for the first bmm.
"""

import os
import numpy as np

import concourse.bacc as bacc
import concourse.mybir as mybir
import concourse.tile as tile
from concourse.bass_utils import run_bass_kernel_spmd
from concourse.masks import make_identity
from concourse.hw_specs import get_activation_tables

F32 = mybir.dt.float32
F32R = mybir.dt.float32r
AX = mybir.AxisListType
ALU = mybir.AluOpType
ACTF = mybir.ActivationFunctionType

NCORES = 8
NB = 128          # total batches
BPC = NB // NCORES  # batches per core
Q = 128
C = 1024
D = 1024
SMOOTH = 20.0

_CACHE = {}


def _build():
    nc = bacc.Bacc("TRN2", target_bir_lowering=False, debug=False,
                   num_devices=NCORES, name="attn_bfan")
    q_in = nc.dram_tensor("query", [BPC, Q, D], F32, kind="ExternalInput")
    c_in = nc.dram_tensor("context", [BPC, C, D], F32, kind="ExternalInput")
    re_out = nc.dram_tensor("re_attn", [BPC, Q, C], F32, kind="ExternalOutput")
    wc_out = nc.dram_tensor("wcontext", [BPC, Q, D], F32, kind="ExternalOutput")

    with tile.TileContext(nc) as tc:
        with (
            tc.tile_pool(name="singles", bufs=1) as singles,
            tc.tile_pool(name="ctxp", bufs=3) as ctxp,
            tc.tile_pool(name="ctxtp", bufs=1) as ctxtp,
            tc.tile_pool(name="qp", bufs=2) as qp,
            tc.tile_pool(name="work", bufs=2) as work,
            tc.tile_pool(name="w1", bufs=1) as w1,
            tc.tile_pool(name="stat", bufs=2) as stat,
            tc.tile_pool(name="ps_a", bufs=1, space="PSUM") as ps_a,
            tc.tile_pool(name="ps_w", bufs=1, space="PSUM") as ps_w,
            tc.tile_pool(name="ps_f", bufs=1, space="PSUM") as ps_f,
            tc.tile_pool(name="ps_tp", bufs=3, space="PSUM") as ps_tp,
        ):
            tab_names = list(get_activation_tables("gen3").keys())
            nc.scalar.add_instruction(mybir.InstLoadActFuncSet(
                name=nc.get_next_instruction_name(),
                act_func_set_id=tab_names.index("natural_log_exp_and_others"),
                ins=[], outs=[]))
            ident = singles.tile([128, 128], F32, tag="ident")
            make_identity(nc, ident[:])
            identr = singles.tile([128, 128], F32R, tag="identr")
            nc.vector.tensor_copy(identr[:], ident[:])
            ones_f = singles.tile([128, 128], F32, tag="ones_f")
            nc.vector.memset(ones_f[:], 1.0)
            ones_r = singles.tile([128, 128], F32R, tag="ones_r")
            nc.vector.tensor_copy(ones_r[:], ones_f[:])
            ln20 = singles.tile([128, 1], F32, tag="ln20")
            nc.vector.memset(ln20[:], float(np.log(SMOOTH)))

            for b in range(BPC):
                # ---- loads (SWDGE cast f32 -> f32r) ----
                ctx = ctxp.tile([128, 8, D], F32R, tag="ctx")
                ctx_src = c_in[b].rearrange("(jc p) d -> p jc d", p=128)
                nc.gpsimd.dma_start(out=ctx[:, 0:4, :], in_=ctx_src[:, 0:4, :])
                nc.gpsimd.dma_start(out=ctx[:, 4:8, :], in_=ctx_src[:, 4:8, :])
                qr = qp.tile([128, D], F32R, tag="qr")
                nc.gpsimd.dma_start(out=qr[:], in_=q_in[b])

                # ---- query^T: 8 PE transposes, 4 per psum bank ----
                qT = qp.tile([128, 8, 128], F32R, tag="qT")
                for g in range(2):
                    tp = ps_tp.tile([128, 512], F32R, tag="tp")
                    for j in range(4):
                        jd = g * 4 + j
                        nc.tensor.transpose(
                            tp[:, j * 128:(j + 1) * 128],
                            qr[:, jd * 128:(jd + 1) * 128], identr[:])
                    if g == 0:
                        nc.vector.tensor_copy(
                            qT[:, 0:4, :].rearrange("p a b -> p (a b)"), tp[:])
                    else:
                        nc.scalar.copy(
                            qT[:, 4:8, :].rearrange("p a b -> p (a b)"), tp[:])

                # ---- context^T: 64 PE transposes, 4 per bank, 16 copies ----
                ctxT = ctxtp.tile([128, 8, C], F32R, tag="ctxT")
                for jd in range(8):
                    for g in range(2):
                        tp = ps_tp.tile([128, 512], F32R, tag="tp")
                        for j in range(4):
                            jc = g * 4 + j
                            nc.tensor.transpose(
                                tp[:, j * 128:(j + 1) * 128],
                                ctx[:, jc, jd * 128:(jd + 1) * 128], identr[:])
                        if (jd * 2 + g) % 8 < 5:
                            nc.vector.tensor_copy(ctxT[:, jd, g * 512:(g + 1) * 512], tp[:])
                        else:
                            nc.scalar.copy(ctxT[:, jd, g * 512:(g + 1) * 512], tp[:])

                # ---- bmm1: attn^T (q, c) accumulated over 8 d-chunks ----
                a0 = ps_a.tile([128, 512], F32, tag="a0")
                a1 = ps_a.tile([128, 512], F32, tag="a1")
                for jd in range(8):
                    st, sp = jd == 0, jd == 7
                    nc.tensor.matmul(a0[:], qT[:, jd, :], ctxT[:, jd, 0:512],
                                     start=st, stop=sp)
                    nc.tensor.matmul(a1[:], qT[:, jd, :], ctxT[:, jd, 512:1024],
                                     start=st, stop=sp)

                # ---- leaky relu via Prelu(alpha=0.1), PSUM -> SBUF ----
                attn = work.tile([128, C], F32, tag="attn")
                nc.scalar.activation(attn[:, 0:512], a0[:], ACTF.Prelu,
                                     bias=0.0, scale=1.0, alpha=0.1)
                nc.scalar.activation(attn[:, 512:1024], a1[:], ACTF.Prelu,
                                     bias=0.0, scale=1.0, alpha=0.1)

                # ---- l2 norm over q: square, ones-matmul (sums over q AND
                #      broadcasts to 128 partitions), sqrt, reciprocal ----
                sq = w1.tile([128, C], F32R, tag="w1a")
                nc.scalar.square(sq[:], attn[:])
                s0 = ps_a.tile([128, 512], F32, tag="a0")
                s1 = ps_a.tile([128, 512], F32, tag="a1")
                nc.tensor.matmul(s0[:], ones_r[:], sq[:, 0:512], start=True, stop=True)
                nc.tensor.matmul(s1[:], ones_r[:], sq[:, 512:1024], start=True, stop=True)
                # 20/sqrt(S) = exp(-0.5*ln(S) + ln 20): keeps ACT on the
                # natural_log_exp table (Exp/Ln/Prelu/Square/Copy), no reloads
                lnS = w1.tile([128, C], F32, tag="w1b")
                nc.scalar.activation(lnS[:, 0:512], s0[:], ACTF.Ln)
                nc.scalar.activation(lnS[:, 512:1024], s1[:], ACTF.Ln)
                rn20 = w1.tile([128, C], F32, tag="w1c")
                nc.scalar.activation(rn20[:], lnS[:], ACTF.Exp,
                                     bias=ln20[:], scale=-0.5)
                u = w1.tile([128, C], F32, tag="w1a")
                nc.vector.tensor_mul(u[:], attn[:], rn20[:])

                # ---- softmax (no max-sub; |u| <= 20) with fused row-sum ----
                pu = work.tile([128, C], F32, tag="pu")
                rs = stat.tile([128, 1], F32, tag="rs")
                nc.scalar.activation(pu[:], u[:], ACTF.Exp,
                                     bias=0.0, scale=1.0, accum_out=rs[:])

                # ---- focal threshold: t = (pu > rs/C) * pu, ts = sum_c t ----
                thr = stat.tile([128, 1], F32, tag="thr")
                nc.scalar.mul(thr[:], rs[:], 1.0 / C)
                t = w1.tile([128, C], F32, tag="w1b")
                ts = stat.tile([128, 1], F32, tag="ts")
                nc.vector.scalar_tensor_tensor(
                    out=t[:], in0=pu[:], scalar=thr[:], in1=pu[:],
                    op0=ALU.is_gt, op1=ALU.mult, accum_out=ts[:])
                rinv = stat.tile([128, 1], F32, tag="rinv")
                nc.vector.reciprocal(rinv[:], ts[:])
                re = work.tile([128, C], F32, tag="re")
                nc.scalar.activation(re[:], t[:], ACTF.Copy, bias=0.0, scale=rinv[:])
                nc.sync.dma_start(out=re_out[b], in_=re[:])

                # ---- re_attn^T for bmm2 (f32 transpose, rounded on eviction) ----
                raT = w1.tile([128, 8, 128], F32R, tag="raT")
                for g in range(2):
                    tpf = ps_f.tile([128, 512], F32, tag="tpf")
                    for j in range(4):
                        jc = g * 4 + j
                        nc.tensor.transpose(
                            tpf[:, j * 128:(j + 1) * 128],
                            re[:, jc * 128:(jc + 1) * 128], ident[:])
                    if g == 0:
                        nc.vector.tensor_copy(
                            raT[:, 0:4, :].rearrange("p a b -> p (a b)"), tpf[:])
                    else:
                        nc.scalar.copy(
                            raT[:, 4:8, :].rearrange("p a b -> p (a b)"), tpf[:])

                # ---- bmm2: wcontext (q, d) accumulated over 8 c-chunks ----
                w0 = ps_w.tile([128, 512], F32, tag="w0")
                w2 = ps_w.tile([128, 512], F32, tag="w1")
                for jc in range(8):
                    st, sp = jc == 0, jc == 7
                    nc.tensor.matmul(w0[:], raT[:, jc, :], ctx[:, jc, 0:512],
                                     start=st, stop=sp)
                    nc.tensor.matmul(w2[:], raT[:, jc, :], ctx[:, jc, 512:1024],
                                     start=st, stop=sp)
                wc = work.tile([128, D], F32, tag="wc")
                nc.scalar.copy(wc[:, 0:512], w0[:])
                nc.vector.tensor_copy(wc[:, 512:1024], w2[:])
                nc.sync.dma_start(out=wc_out[b], in_=wc[:])

    nc.compile()
    return nc


def kernel(query: np.ndarray, context: np.ndarray):
    query = np.ascontiguousarray(query, dtype=np.float32)
    context = np.ascontiguousarray(context, dtype=np.float32)
    assert query.shape == (NB, Q, D) and context.shape == (NB, C, D)

    if "nc" not in _CACHE:
        _CACHE["nc"] = _build()
    nc = _CACHE["nc"]

    in_maps = []
    for k in range(NCORES):
        sl = slice(k * BPC, (k + 1) * BPC)
        in_maps.append({"query": query[sl], "context": context[sl]})

    trace = os.environ.get("KERNEL_TRACE", "0") == "1"
    res = run_bass_kernel_spmd(nc, in_maps, core_ids=list(range(NCORES)),
                               trace=trace)
    _CACHE["last_res"] = res

    re_attn = np.concatenate([r["re_attn"] for r in res.results], axis=0)
    wcontext = np.concatenate([r["wcontext"] for r in res.results], axis=0)
    return query, wcontext, re_attn
